# revision 15
# baseline (speedup 1.0000x reference)
"""Trainium2 Bass kernel for nn_DifferentiableArray (retrieval_knn).

Reference computation (B=2048, N=65536, D=64):
    query  = normalized_indices @ key_weight.T          # [B, D]
    sims   = query @ position_embeddings.T              # [B, N]
    attn   = softmax(sims, axis=-1)                     # [B, N]
    output = sum(attn * values_batch, -1, keepdims)     # [B, 1]
    return output, attn, sims

Key algebraic identity: sims[b, n] = ni[b] * s[n] with
    s = position_embeddings @ key_weight  (shape [N]),
a rank-1 outer product. Per core (batch-sharded 8 ways, 256 rows of B):
    pass 1: stream values, e = exp(ni*s) per chunk on ACT (row-sums Z via
            ACT accum), weighted value sum: multiply on DVE/Pool
            (alternating), reduce on DVE.
    pass 2: recompute sims on PE (K=1 fp32r matmul ni_row x s_chunk),
            copy sims PSUM->SBUF on ACT, attn = exp(sims - ln Z) on ACT
            with -ln(Z) folded into the activation bias.
Pass 2 of batch-tile 0 is interleaved with pass 1 of batch-tile 1 so the
write-heavy and read-heavy streams share the DMA engines. Softmax
max-subtraction is skipped: ni in [0,1), |s| <= ~40 so exp cannot
overflow fp32. Matmul operands are fp32r (full-rate on PE; ~2^-12 input
rounding, well within tolerance). Big tensors move as 2MB DMAs (16KB
per partition line); values-in dispatches on the ACT HWDGE queue,
everything else on SP, so neither queue head-blocks the other.
"""

import sys

sys.path.insert(0, "/opt/trn_rl_repo")

import numpy as np

import concourse.bacc as bacc
import concourse.mybir as mybir
import concourse.tile as tile
from concourse.bass_utils import run_bass_kernel_spmd

F32 = mybir.dt.float32
F32R = mybir.dt.float32r
AF = mybir.ActivationFunctionType
ALU = mybir.AluOpType
AX = mybir.AxisListType

B = 2048
N = 65536
D = 64
N_CORES = 8
B_LOC = B // N_CORES          # 256 rows per core
PB = 128                      # batch rows per partition tile
N_BT = B_LOC // PB            # 2 batch tiles per core
CF = 2048                     # compute chunk (4 PSUM banks)
DG = 4096                     # DMA granule (16KB per partition line)
NG = N // DG                  # 16 DMA groups
CPG = DG // CF                # 2 compute chunks per DMA group
NCH = N // CF                 # 32 compute chunks
MMF = 512                     # matmul moving free max
SROW = N // MMF               # 128 rows of s_2d

_compiled = None


def _build():
    nc = bacc.Bacc("TRN2", target_bir_lowering=False, debug=False,
                   num_devices=N_CORES)

    v_dram = nc.dram_tensor("values", [B_LOC, N], F32, kind="ExternalInput")
    ni_dram = nc.dram_tensor("nidx", [B_LOC, 1], F32, kind="ExternalInput")
    pe_dram = nc.dram_tensor("posemb", [N, D], F32, kind="ExternalInput")
    kw_dram = nc.dram_tensor("keyw", [D, 1], F32, kind="ExternalInput")
    out_dram = nc.dram_tensor("out", [B_LOC, 1], F32, kind="ExternalOutput")
    att_dram = nc.dram_tensor("attn", [B_LOC, N], F32, kind="ExternalOutput")
    sim_dram = nc.dram_tensor("sims", [B_LOC, N], F32, kind="ExternalOutput")
    s_dram = nc.dram_tensor("s_scratch", [N], F32R)

    with tile.TileContext(nc) as tc:
        with tc.tile_pool(name="const", bufs=1) as cpool:
            # s_2d[p, f] = s[p*MMF + f]
            s_2d = cpool.tile([SROW, MMF], F32)
            s_2dr = cpool.tile([SROW, MMF], F32R)
            kw_row = cpool.tile([1, D], F32)
            kwb = cpool.tile([128, D], F32)

            nc.sync.dma_start(kw_row[:], kw_dram[:].rearrange("d one -> one d"))
            nc.gpsimd.partition_broadcast(kwb[:], kw_row[:])

            # prolog: s = pe @ kw via mul + reduce over D
            pe_view = pe_dram[:].rearrange("(p f) d -> p f d", p=SROW)
            FQ = MMF // 4
            kwb_b = kwb[:].unsqueeze(1).broadcast_to([SROW, FQ, D])
            with tc.tile_pool(name="prolog", bufs=2) as ppool:
                for q in range(4):
                    pe_t = ppool.tile([SROW, FQ, D], F32)
                    nc.sync.dma_start(pe_t[:], pe_view[:, q * FQ:(q + 1) * FQ, :])
                    prod = ppool.tile([SROW, FQ, D], F32)
                    mul_eng = nc.vector if q % 2 == 0 else nc.gpsimd
                    mul_eng.tensor_tensor(prod[:], pe_t[:], kwb_b, ALU.mult)
                    nc.vector.reduce_sum(s_2d[:, q * FQ:(q + 1) * FQ], prod[:],
                                         axis=AX.X)
            # round to fp32r; round-trip via DRAM so chunk slices can be
            # staged on partition 0 (matmul operands need base partition 0)
            nc.vector.tensor_copy(s_2dr[:], s_2d[:])
            nc.sync.dma_start(s_dram[:].rearrange("(p f) -> p f", p=SROW),
                              s_2dr[:])

            with (
                tc.tile_pool(name="bt", bufs=2) as bpool,
                tc.tile_pool(name="vpool", bufs=3) as vpool,
                tc.tile_pool(name="work", bufs=2) as wpool,
                tc.tile_pool(name="stage", bufs=2) as spool,
                tc.tile_pool(name="psum", bufs=2, space="PSUM") as pspool,
            ):
                ni_view = ni_dram[:].rearrange("(t p) one -> t (one p)", p=PB)

                def bt_begin(bt):
                    ni_row = bpool.tile([1, PB], F32)
                    nc.sync.dma_start(ni_row[:], ni_view[bt:bt + 1, :])
                    ni_r = bpool.tile([1, PB], F32R)
                    nc.vector.tensor_copy(ni_r[:], ni_row[:])
                    z_cols = bpool.tile([PB, NCH], F32)
                    w_cols = bpool.tile([PB, NCH], F32)
                    return dict(bt=bt, r0=bt * PB, ni_r=ni_r,
                                z_cols=z_cols, w_cols=w_cols)

                def stage_s(g):
                    m0 = g * DG
                    s_st = spool.tile([1, DG], F32R)
                    nc.scalar.dma_start(s_st[:], s_dram[m0:m0 + DG].unsqueeze(0))
                    return s_st

                def pass1_group(st, g, s_st=None):
                    m0 = g * DG
                    v_t = vpool.tile([PB, DG], F32)
                    nc.scalar.dma_start(
                        v_t[:], v_dram[st["r0"]:st["r0"] + PB, m0:m0 + DG])
                    if s_st is None:
                        s_st = stage_s(g)
                    for h in range(CPG):
                        c = g * CPG + h
                        sim_p = pspool.tile([PB, CF], F32)
                        for j in range(4):
                            f0 = h * CF + j * MMF
                            nc.tensor.matmul(sim_p[:, j * MMF:(j + 1) * MMF],
                                             st["ni_r"][:], s_st[:, f0:f0 + MMF],
                                             start=True, stop=True)
                        e_t = wpool.tile([PB, CF], F32)
                        nc.scalar.activation(e_t[:], sim_p[:], AF.Exp,
                                             accum_out=st["z_cols"][:, c:c + 1])
                        ev_t = wpool.tile([PB, CF], F32)
                        mul_eng = nc.vector if h == 0 else nc.gpsimd
                        mul_eng.tensor_tensor(ev_t[:], e_t[:],
                                              v_t[:, h * CF:(h + 1) * CF],
                                              ALU.mult)
                        nc.vector.reduce_sum(st["w_cols"][:, c:c + 1], ev_t[:],
                                             axis=AX.X)

                def bt_mid(st):
                    zsum = bpool.tile([PB, 1], F32)
                    wsum = bpool.tile([PB, 1], F32)
                    rz = bpool.tile([PB, 1], F32)
                    outc = bpool.tile([PB, 1], F32)
                    lnz = bpool.tile([PB, 1], F32)
                    nbias = bpool.tile([PB, 1], F32)
                    nc.vector.reduce_sum(zsum[:], st["z_cols"][:], axis=AX.X)
                    nc.vector.reduce_sum(wsum[:], st["w_cols"][:], axis=AX.X)
                    nc.vector.reciprocal(rz[:], zsum[:])
                    nc.vector.tensor_tensor(outc[:], wsum[:], rz[:], ALU.mult)
                    nc.sync.dma_start(out_dram[st["r0"]:st["r0"] + PB, :],
                                      outc[:])
                    nc.scalar.activation(lnz[:], zsum[:], AF.Ln)
                    nc.vector.tensor_scalar_mul(nbias[:], lnz[:], -1.0)
                    st["nbias"] = nbias

                def pass2_group(st, g, s_st=None):
                    m0 = g * DG
                    if s_st is None:
                        s_st = stage_s(g)
                    att_st = spool.tile([PB, DG], F32)
                    sim_st = spool.tile([PB, DG], F32)
                    for h in range(CPG):
                        sim_p = pspool.tile([PB, CF], F32)
                        for j in range(4):
                            f0 = h * CF + j * MMF
                            nc.tensor.matmul(sim_p[:, j * MMF:(j + 1) * MMF],
                                             st["ni_r"][:], s_st[:, f0:f0 + MMF],
                                             start=True, stop=True)
                        nc.scalar.copy(sim_st[:, h * CF:(h + 1) * CF], sim_p[:])
                        nc.scalar.activation(att_st[:, h * CF:(h + 1) * CF],
                                             sim_p[:], AF.Exp,
                                             bias=st["nbias"][:])
                    nc.sync.dma_start(
                        sim_dram[st["r0"]:st["r0"] + PB, m0:m0 + DG], sim_st[:])
                    nc.sync.dma_start(
                        att_dram[st["r0"]:st["r0"] + PB, m0:m0 + DG], att_st[:])

                # phase A: pass 1 of batch-tile 0
                st0 = bt_begin(0)
                for g in range(NG):
                    pass1_group(st0, g)
                bt_mid(st0)
                # phase B: pass 2 of bt0 interleaved with pass 1 of bt1
                st1 = bt_begin(1)
                for g in range(NG):
                    s_sh = stage_s(g)
                    pass2_group(st0, g, s_sh)
                    pass1_group(st1, g, s_sh)
                bt_mid(st1)
                # phase C: pass 2 of bt1
                for g in range(NG):
                    pass2_group(st1, g)

    nc.compile()
    return nc


def _get_compiled():
    global _compiled
    if _compiled is None:
        _compiled = _build()
    return _compiled


def kernel(values_batch, normalized_indices, position_embeddings, key_weight):
    values_batch = np.ascontiguousarray(values_batch, dtype=np.float32)
    normalized_indices = np.ascontiguousarray(normalized_indices, dtype=np.float32)
    position_embeddings = np.ascontiguousarray(position_embeddings, dtype=np.float32)
    key_weight = np.ascontiguousarray(key_weight, dtype=np.float32)

    nc = _get_compiled()
    in_maps = []
    for i in range(N_CORES):
        r0 = i * B_LOC
        in_maps.append({
            "values": values_batch[r0:r0 + B_LOC],
            "nidx": normalized_indices[r0:r0 + B_LOC],
            "posemb": position_embeddings,
            "keyw": key_weight,
        })
    res = run_bass_kernel_spmd(nc, in_maps, list(range(N_CORES)))
    output = np.concatenate([res.results[i]["out"] for i in range(N_CORES)], axis=0)
    attention = np.concatenate([res.results[i]["attn"] for i in range(N_CORES)], axis=0)
    similarities = np.concatenate([res.results[i]["sims"] for i in range(N_CORES)], axis=0)
    return output, attention, similarities


# revision 19
# speedup vs baseline: 1.0655x; 1.0655x over previous
"""Trainium2 Bass kernel for nn_DifferentiableArray (retrieval_knn).

Reference computation (B=2048, N=65536, D=64):
    query  = normalized_indices @ key_weight.T          # [B, D]
    sims   = query @ position_embeddings.T              # [B, N]
    attn   = softmax(sims, axis=-1)                     # [B, N]
    output = sum(attn * values_batch, -1, keepdims)     # [B, 1]
    return output, attn, sims

Key algebraic identity: sims[b, n] = ni[b] * s[n] with
    s = position_embeddings @ key_weight  (shape [N]),
a rank-1 outer product. Per core (batch-sharded 8 ways, 256 rows of B):
    pass 1: stream values, e = exp(ni*s) per chunk on ACT (row-sums Z via
            ACT accum), weighted value sum: multiply on DVE/Pool
            (alternating), reduce on DVE.
    pass 2: recompute sims on PE (K=1 fp32r matmul ni_row x s_chunk),
            copy sims PSUM->SBUF on ACT, attn = exp(sims - ln Z) on ACT
            with -ln(Z) folded into the activation bias.
Pass 2 of batch-tile 0 is interleaved with pass 1 of batch-tile 1 so the
write-heavy and read-heavy streams share the DMA engines. Softmax
max-subtraction is skipped: ni in [0,1), |s| <= ~40 so exp cannot
overflow fp32. Matmul operands are fp32r (full-rate on PE; ~2^-12 input
rounding, well within tolerance). Big tensors move as 2MB DMAs (16KB
per partition line); values-in dispatches on the ACT HWDGE queue,
everything else on SP, so neither queue head-blocks the other.
"""

import sys

sys.path.insert(0, "/opt/trn_rl_repo")

import numpy as np

import concourse.bacc as bacc
import concourse.mybir as mybir
import concourse.tile as tile
from concourse.bass_utils import run_bass_kernel_spmd

F32 = mybir.dt.float32
F32R = mybir.dt.float32r
AF = mybir.ActivationFunctionType
ALU = mybir.AluOpType
AX = mybir.AxisListType

B = 2048
N = 65536
D = 64
N_CORES = 8
B_LOC = B // N_CORES          # 256 rows per core
PB = 128                      # batch rows per partition tile
N_BT = B_LOC // PB            # 2 batch tiles per core
CF = 2048                     # compute chunk (4 PSUM banks)
DG = 4096                     # DMA granule (16KB per partition line)
NG = N // DG                  # 16 DMA groups
CPG = DG // CF                # 2 compute chunks per DMA group
NCH = N // CF                 # 32 compute chunks
MMF = 512                     # matmul moving free max
SROW = N // MMF               # 128 rows of s_2d

_compiled = None


def _build():
    nc = bacc.Bacc("TRN2", target_bir_lowering=False, debug=False,
                   num_devices=N_CORES)

    v_dram = nc.dram_tensor("values", [B_LOC, N], F32, kind="ExternalInput")
    ni_dram = nc.dram_tensor("nidx", [B_LOC, 1], F32, kind="ExternalInput")
    pe_dram = nc.dram_tensor("peshard", [N // N_CORES, D], F32,
                             kind="ExternalInput")
    kw_dram = nc.dram_tensor("keyw", [D, 1], F32, kind="ExternalInput")
    out_dram = nc.dram_tensor("out", [B_LOC, 1], F32, kind="ExternalOutput")
    att_dram = nc.dram_tensor("attn", [B_LOC, N], F32, kind="ExternalOutput")
    sim_dram = nc.dram_tensor("sims", [B_LOC, N], F32, kind="ExternalOutput")
    ag_in = nc.dram_tensor("s_shard", [N // N_CORES], F32R)
    s_dram = nc.dram_tensor("s_full", [N], F32R)

    with tile.TileContext(nc) as tc:
        with tc.tile_pool(name="const", bufs=1) as cpool:
            # s_2d[p, f] = s[p*MMF + f]
            s_2d = cpool.tile([SROW, MMF], F32)
            s_2dr = cpool.tile([SROW, MMF], F32R)
            kw_row = cpool.tile([1, D], F32)
            kwb = cpool.tile([128, D], F32)

            nc.sync.dma_start(kw_row[:], kw_dram[:].rearrange("d one -> one d"))
            nc.gpsimd.partition_broadcast(kwb[:], kw_row[:])

            # prolog: this core computes s for its 1/8 shard of positions
            # (host feeds pe rows [rank*N/8, (rank+1)*N/8)), then AllGather
            # concatenates the shards in rank order == position order.
            FSH = N // N_CORES // 128     # 64 positions per partition
            pe_view = pe_dram[:].rearrange("(p f) d -> p f d", p=128)
            kwb_b = kwb[:].unsqueeze(1).broadcast_to([128, FSH, D])
            with tc.tile_pool(name="prolog", bufs=1) as ppool:
                pe_t = ppool.tile([128, FSH, D], F32)
                nc.sync.dma_start(pe_t[:], pe_view[:])
                prod = ppool.tile([128, FSH, D], F32)
                nc.vector.tensor_tensor(prod[:], pe_t[:], kwb_b, ALU.mult)
                nc.vector.reduce_sum(s_2d[:, :FSH], prod[:], axis=AX.X)
                nc.vector.tensor_copy(s_2dr[:, :FSH], s_2d[:, :FSH])
                nc.sync.dma_start(
                    ag_in[:].rearrange("(p f) -> p f", p=128),
                    s_2dr[:, :FSH])
                nc.gpsimd.collective_compute(
                    "AllGather", ALU.bypass,
                    replica_groups=[list(range(N_CORES))],
                    ins=[ag_in[:]], outs=[s_dram[:]])

            with (
                tc.tile_pool(name="bt", bufs=2) as bpool,
                tc.tile_pool(name="vpool", bufs=3) as vpool,
                tc.tile_pool(name="work", bufs=2) as wpool,
                tc.tile_pool(name="stage", bufs=2) as spool,
                tc.tile_pool(name="psum", bufs=2, space="PSUM") as pspool,
            ):
                ni_view = ni_dram[:].rearrange("(t p) one -> t (one p)", p=PB)

                def bt_begin(bt):
                    ni_row = bpool.tile([1, PB], F32)
                    nc.sync.dma_start(ni_row[:], ni_view[bt:bt + 1, :])
                    ni_r = bpool.tile([1, PB], F32R)
                    nc.vector.tensor_copy(ni_r[:], ni_row[:])
                    z_cols = bpool.tile([PB, NCH], F32)
                    w_cols = bpool.tile([PB, NCH], F32)
                    return dict(bt=bt, r0=bt * PB, ni_r=ni_r,
                                z_cols=z_cols, w_cols=w_cols)

                def stage_s(g):
                    m0 = g * DG
                    s_st = spool.tile([1, DG], F32R)
                    nc.scalar.dma_start(s_st[:], s_dram[m0:m0 + DG].unsqueeze(0))
                    return s_st

                def pass1_group(st, g, s_st=None):
                    m0 = g * DG
                    v_t = vpool.tile([PB, DG], F32)
                    nc.scalar.dma_start(
                        v_t[:], v_dram[st["r0"]:st["r0"] + PB, m0:m0 + DG])
                    if s_st is None:
                        s_st = stage_s(g)
                    for h in range(CPG):
                        c = g * CPG + h
                        sim_p = pspool.tile([PB, CF], F32)
                        for j in range(4):
                            f0 = h * CF + j * MMF
                            nc.tensor.matmul(sim_p[:, j * MMF:(j + 1) * MMF],
                                             st["ni_r"][:], s_st[:, f0:f0 + MMF],
                                             start=True, stop=True)
                        e_t = wpool.tile([PB, CF], F32)
                        nc.scalar.activation(e_t[:], sim_p[:], AF.Exp,
                                             accum_out=st["z_cols"][:, c:c + 1])
                        ev_t = wpool.tile([PB, CF], F32)
                        mul_eng = nc.vector if h == 0 else nc.gpsimd
                        mul_eng.tensor_tensor(ev_t[:], e_t[:],
                                              v_t[:, h * CF:(h + 1) * CF],
                                              ALU.mult)
                        nc.vector.reduce_sum(st["w_cols"][:, c:c + 1], ev_t[:],
                                             axis=AX.X)

                def bt_mid(st):
                    zsum = bpool.tile([PB, 1], F32)
                    wsum = bpool.tile([PB, 1], F32)
                    rz = bpool.tile([PB, 1], F32)
                    outc = bpool.tile([PB, 1], F32)
                    lnz = bpool.tile([PB, 1], F32)
                    nbias = bpool.tile([PB, 1], F32)
                    nc.vector.reduce_sum(zsum[:], st["z_cols"][:], axis=AX.X)
                    nc.vector.reduce_sum(wsum[:], st["w_cols"][:], axis=AX.X)
                    nc.vector.reciprocal(rz[:], zsum[:])
                    nc.vector.tensor_tensor(outc[:], wsum[:], rz[:], ALU.mult)
                    nc.sync.dma_start(out_dram[st["r0"]:st["r0"] + PB, :],
                                      outc[:])
                    nc.scalar.activation(lnz[:], zsum[:], AF.Ln)
                    nc.vector.tensor_scalar_mul(nbias[:], lnz[:], -1.0)
                    st["nbias"] = nbias

                def pass2_group(st, g, s_st=None):
                    m0 = g * DG
                    if s_st is None:
                        s_st = stage_s(g)
                    att_st = spool.tile([PB, DG], F32)
                    sim_st = spool.tile([PB, DG], F32)
                    for h in range(CPG):
                        sim_p = pspool.tile([PB, CF], F32)
                        for j in range(4):
                            f0 = h * CF + j * MMF
                            nc.tensor.matmul(sim_p[:, j * MMF:(j + 1) * MMF],
                                             st["ni_r"][:], s_st[:, f0:f0 + MMF],
                                             start=True, stop=True)
                        nc.scalar.copy(sim_st[:, h * CF:(h + 1) * CF], sim_p[:])
                        nc.scalar.activation(att_st[:, h * CF:(h + 1) * CF],
                                             sim_p[:], AF.Exp,
                                             bias=st["nbias"][:])
                    nc.sync.dma_start(
                        sim_dram[st["r0"]:st["r0"] + PB, m0:m0 + DG], sim_st[:])
                    nc.sync.dma_start(
                        att_dram[st["r0"]:st["r0"] + PB, m0:m0 + DG], att_st[:])

                # phase A: pass 1 of batch-tile 0
                st0 = bt_begin(0)
                for g in range(NG):
                    pass1_group(st0, g)
                bt_mid(st0)
                # phase B: pass 2 of bt0 interleaved with pass 1 of bt1
                st1 = bt_begin(1)
                for g in range(NG):
                    s_sh = stage_s(g)
                    pass2_group(st0, g, s_sh)
                    pass1_group(st1, g, s_sh)
                bt_mid(st1)
                # phase C: pass 2 of bt1
                for g in range(NG):
                    pass2_group(st1, g)

    nc.compile()
    return nc


def _get_compiled():
    global _compiled
    if _compiled is None:
        _compiled = _build()
    return _compiled


def kernel(values_batch, normalized_indices, position_embeddings, key_weight):
    values_batch = np.ascontiguousarray(values_batch, dtype=np.float32)
    normalized_indices = np.ascontiguousarray(normalized_indices, dtype=np.float32)
    position_embeddings = np.ascontiguousarray(position_embeddings, dtype=np.float32)
    key_weight = np.ascontiguousarray(key_weight, dtype=np.float32)

    nc = _get_compiled()
    in_maps = []
    for i in range(N_CORES):
        r0 = i * B_LOC
        n0 = i * (N // N_CORES)
        in_maps.append({
            "values": values_batch[r0:r0 + B_LOC],
            "nidx": normalized_indices[r0:r0 + B_LOC],
            "peshard": position_embeddings[n0:n0 + N // N_CORES],
            "keyw": key_weight,
        })
    res = run_bass_kernel_spmd(nc, in_maps, list(range(N_CORES)))
    output = np.concatenate([res.results[i]["out"] for i in range(N_CORES)], axis=0)
    attention = np.concatenate([res.results[i]["attn"] for i in range(N_CORES)], axis=0)
    similarities = np.concatenate([res.results[i]["sims"] for i in range(N_CORES)], axis=0)
    return output, attention, similarities


# revision 25
# speedup vs baseline: 1.0858x; 1.0191x over previous
"""Trainium2 Bass kernel for nn_DifferentiableArray (retrieval_knn).

Reference computation (B=2048, N=65536, D=64):
    query  = normalized_indices @ key_weight.T          # [B, D]
    sims   = query @ position_embeddings.T              # [B, N]
    attn   = softmax(sims, axis=-1)                     # [B, N]
    output = sum(attn * values_batch, -1, keepdims)     # [B, 1]
    return output, attn, sims

Key algebraic identity: sims[b, n] = ni[b] * s[n] with
    s = position_embeddings @ key_weight  (shape [N]),
a rank-1 outer product. Per core (batch-sharded 8 ways, 256 rows of B):
    pass 1: stream values, e = exp(ni*s) per chunk on ACT (row-sums Z via
            ACT accum), weighted value sum: multiply on DVE/Pool
            (alternating), reduce on DVE.
    pass 2: recompute sims on PE (K=1 fp32r matmul ni_row x s_chunk),
            copy sims PSUM->SBUF on ACT, attn = exp(sims - ln Z) on ACT
            with -ln(Z) folded into the activation bias.
Pass 2 of batch-tile 0 is interleaved with pass 1 of batch-tile 1 so the
write-heavy and read-heavy streams share the DMA engines. Softmax
max-subtraction is skipped: ni in [0,1), |s| <= ~40 so exp cannot
overflow fp32. Matmul operands are fp32r (full-rate on PE; ~2^-12 input
rounding, well within tolerance). Big tensors move as 2MB DMAs (16KB
per partition line); values-in dispatches on the ACT HWDGE queue,
everything else on SP, so neither queue head-blocks the other.
"""

import sys

sys.path.insert(0, "/opt/trn_rl_repo")

import numpy as np

import concourse.bacc as bacc
import concourse.mybir as mybir
import concourse.tile as tile
from concourse.bass_utils import run_bass_kernel_spmd

F32 = mybir.dt.float32
F32R = mybir.dt.float32r
AF = mybir.ActivationFunctionType
ALU = mybir.AluOpType
AX = mybir.AxisListType

B = 2048
N = 65536
D = 64
N_CORES = 8
B_LOC = B // N_CORES          # 256 rows per core
PB = 128                      # batch rows per partition tile
N_BT = B_LOC // PB            # 2 batch tiles per core
CF = 2048                     # compute chunk (4 PSUM banks)
DG = 4096                     # DMA granule (16KB per partition line)
NG = N // DG                  # 16 DMA groups
CPG = DG // CF                # 2 compute chunks per DMA group
NCH = N // CF                 # 32 compute chunks
MMF = 512                     # matmul moving free max
SROW = N // MMF               # 128 rows of s_2d

_compiled = None


def _build():
    nc = bacc.Bacc("TRN2", target_bir_lowering=False, debug=False,
                   num_devices=N_CORES)

    v_dram = nc.dram_tensor("values", [B_LOC, N], F32, kind="ExternalInput")
    ni_dram = nc.dram_tensor("nidx", [B_LOC, 1], F32, kind="ExternalInput")
    pe_dram = nc.dram_tensor("peshard", [N // N_CORES, D], F32,
                             kind="ExternalInput")
    kw_dram = nc.dram_tensor("keyw", [D, 1], F32, kind="ExternalInput")
    out_dram = nc.dram_tensor("out", [B_LOC, 1], F32, kind="ExternalOutput")
    att_dram = nc.dram_tensor("attn", [B_LOC, N], F32, kind="ExternalOutput")
    sim_dram = nc.dram_tensor("sims", [B_LOC, N], F32, kind="ExternalOutput")
    ag_in = nc.dram_tensor("s_shard", [N // N_CORES], F32R)
    s_dram = nc.dram_tensor("s_full", [N], F32R)

    with tile.TileContext(nc) as tc:
        with tc.tile_pool(name="const", bufs=1) as cpool:
            # s_2d[p, f] = s[p*MMF + f]
            s_2d = cpool.tile([SROW, MMF], F32)
            s_2dr = cpool.tile([SROW, MMF], F32R)
            kw_row = cpool.tile([1, D], F32)
            kwb = cpool.tile([128, D], F32)

            nc.sync.dma_start(kw_row[:], kw_dram[:].rearrange("d one -> one d"))
            nc.gpsimd.partition_broadcast(kwb[:], kw_row[:])

            # prolog: this core computes s for its 1/8 shard of positions
            # (host feeds pe rows [rank*N/8, (rank+1)*N/8)), then AllGather
            # concatenates the shards in rank order == position order.
            FSH = N // N_CORES // 128     # 64 positions per partition
            pe_view = pe_dram[:].rearrange("(p f) d -> p f d", p=128)
            kwb_b = kwb[:].unsqueeze(1).broadcast_to([128, FSH, D])
            with tc.tile_pool(name="prolog", bufs=1) as ppool:
                pe_t = ppool.tile([128, FSH, D], F32)
                nc.sync.dma_start(pe_t[:], pe_view[:])
                prod = ppool.tile([128, FSH, D], F32)
                nc.vector.tensor_tensor(prod[:], pe_t[:], kwb_b, ALU.mult)
                nc.vector.reduce_sum(s_2d[:, :FSH], prod[:], axis=AX.X)
                nc.vector.tensor_copy(s_2dr[:, :FSH], s_2d[:, :FSH])
                nc.sync.dma_start(
                    ag_in[:].rearrange("(p f) -> p f", p=128),
                    s_2dr[:, :FSH])
                nc.gpsimd.collective_compute(
                    "AllGather", ALU.bypass,
                    replica_groups=[list(range(N_CORES))],
                    ins=[ag_in[:]], outs=[s_dram[:]])

            with (
                tc.tile_pool(name="bt", bufs=2) as bpool,
                tc.tile_pool(name="vpool", bufs=4) as vpool,
                tc.tile_pool(name="work", bufs=2) as wpool,
                tc.tile_pool(name="stage", bufs=2) as spool,
                tc.tile_pool(name="psum", bufs=2, space="PSUM") as pspool,
            ):
                ni_view = ni_dram[:].rearrange("(t p) one -> t (one p)", p=PB)

                def bt_begin(bt):
                    ni_row = bpool.tile([1, PB], F32)
                    nc.sync.dma_start(ni_row[:], ni_view[bt:bt + 1, :])
                    ni_r = bpool.tile([1, PB], F32R)
                    nc.vector.tensor_copy(ni_r[:], ni_row[:])
                    z_cols = bpool.tile([PB, NCH], F32)
                    w_cols = bpool.tile([PB, NCH], F32)
                    return dict(bt=bt, r0=bt * PB, ni_r=ni_r,
                                z_cols=z_cols, w_cols=w_cols)

                def stage_s(g):
                    m0 = g * DG
                    s_st = spool.tile([1, DG], F32R)
                    nc.gpsimd.dma_start(s_st[:], s_dram[m0:m0 + DG].unsqueeze(0))
                    return s_st

                def pass1_group(st, g, s_st=None):
                    m0 = g * DG
                    v_t = vpool.tile([PB, DG], F32)
                    nc.scalar.dma_start(
                        v_t[:], v_dram[st["r0"]:st["r0"] + PB, m0:m0 + DG])
                    if s_st is None:
                        s_st = stage_s(g)
                    for h in range(CPG):
                        c = g * CPG + h
                        sim_p = pspool.tile([PB, CF], F32)
                        for j in range(4):
                            f0 = h * CF + j * MMF
                            nc.tensor.matmul(sim_p[:, j * MMF:(j + 1) * MMF],
                                             st["ni_r"][:], s_st[:, f0:f0 + MMF],
                                             start=True, stop=True)
                        e_t = wpool.tile([PB, CF], F32)
                        nc.scalar.activation(e_t[:], sim_p[:], AF.Exp,
                                             accum_out=st["z_cols"][:, c:c + 1])
                        ev_t = wpool.tile([PB, CF], F32)
                        mul_eng = nc.vector if h == 0 else nc.gpsimd
                        mul_eng.tensor_tensor(ev_t[:], e_t[:],
                                              v_t[:, h * CF:(h + 1) * CF],
                                              ALU.mult)
                        nc.vector.reduce_sum(st["w_cols"][:, c:c + 1], ev_t[:],
                                             axis=AX.X)

                def bt_mid(st):
                    zsum = bpool.tile([PB, 1], F32)
                    wsum = bpool.tile([PB, 1], F32)
                    rz = bpool.tile([PB, 1], F32)
                    outc = bpool.tile([PB, 1], F32)
                    lnz = bpool.tile([PB, 1], F32)
                    nbias = bpool.tile([PB, 1], F32)
                    nc.vector.reduce_sum(zsum[:], st["z_cols"][:], axis=AX.X)
                    nc.vector.reduce_sum(wsum[:], st["w_cols"][:], axis=AX.X)
                    nc.vector.reciprocal(rz[:], zsum[:])
                    nc.vector.tensor_tensor(outc[:], wsum[:], rz[:], ALU.mult)
                    nc.sync.dma_start(out_dram[st["r0"]:st["r0"] + PB, :],
                                      outc[:])
                    nc.scalar.activation(lnz[:], zsum[:], AF.Ln)
                    nc.vector.tensor_scalar_mul(nbias[:], lnz[:], -1.0)
                    st["nbias"] = nbias

                def pass2_group(st, g, s_st=None):
                    m0 = g * DG
                    if s_st is None:
                        s_st = stage_s(g)
                    att_st = spool.tile([PB, DG], F32)
                    sim_st = spool.tile([PB, DG], F32)
                    for h in range(CPG):
                        sim_p = pspool.tile([PB, CF], F32)
                        for j in range(4):
                            f0 = h * CF + j * MMF
                            nc.tensor.matmul(sim_p[:, j * MMF:(j + 1) * MMF],
                                             st["ni_r"][:], s_st[:, f0:f0 + MMF],
                                             start=True, stop=True)
                        nc.scalar.copy(sim_st[:, h * CF:(h + 1) * CF], sim_p[:])
                        nc.scalar.activation(att_st[:, h * CF:(h + 1) * CF],
                                             sim_p[:], AF.Exp,
                                             bias=st["nbias"][:])
                    nc.sync.dma_start(
                        sim_dram[st["r0"]:st["r0"] + PB, m0:m0 + DG], sim_st[:])
                    nc.sync.dma_start(
                        att_dram[st["r0"]:st["r0"] + PB, m0:m0 + DG], att_st[:])

                # phase A: pass 1 of batch-tile 0
                st0 = bt_begin(0)
                for g in range(NG):
                    pass1_group(st0, g)
                bt_mid(st0)
                # phase B: pass 2 of bt0 interleaved with pass 1 of bt1
                st1 = bt_begin(1)
                for g in range(NG):
                    s_sh = stage_s(g)
                    pass2_group(st0, g, s_sh)
                    pass1_group(st1, g, s_sh)
                bt_mid(st1)
                # phase C: pass 2 of bt1
                for g in range(NG):
                    pass2_group(st1, g)

    nc.compile()
    return nc


def _get_compiled():
    global _compiled
    if _compiled is None:
        _compiled = _build()
    return _compiled


def kernel(values_batch, normalized_indices, position_embeddings, key_weight):
    values_batch = np.ascontiguousarray(values_batch, dtype=np.float32)
    normalized_indices = np.ascontiguousarray(normalized_indices, dtype=np.float32)
    position_embeddings = np.ascontiguousarray(position_embeddings, dtype=np.float32)
    key_weight = np.ascontiguousarray(key_weight, dtype=np.float32)

    nc = _get_compiled()
    in_maps = []
    for i in range(N_CORES):
        r0 = i * B_LOC
        n0 = i * (N // N_CORES)
        in_maps.append({
            "values": values_batch[r0:r0 + B_LOC],
            "nidx": normalized_indices[r0:r0 + B_LOC],
            "peshard": position_embeddings[n0:n0 + N // N_CORES],
            "keyw": key_weight,
        })
    res = run_bass_kernel_spmd(nc, in_maps, list(range(N_CORES)))
    output = np.concatenate([res.results[i]["out"] for i in range(N_CORES)], axis=0)
    attention = np.concatenate([res.results[i]["attn"] for i in range(N_CORES)], axis=0)
    similarities = np.concatenate([res.results[i]["sims"] for i in range(N_CORES)], axis=0)
    return output, attention, similarities
